# revision 22
# baseline (speedup 1.0000x reference)
"""DBSCAN (eps=22, min_samples=5) on X[8192, 256] float32, distributed
across 8 TRN2 NeuronCores via Bass/Tile.

Math (mirrors the jax reference):
  d2[i,j] = ||x_i||^2 + ||x_j||^2 - 2 (X X^T)[i,j]
  adj     = d2 <= eps^2
  core_i  = rowsum(adj) >= min_samples
  comp    = min-index label propagation over the core-core eps-graph
  labels  = component ids in scan order; border points attach to the
            min-index core neighbor; rest are noise (-1).

Sharding: core m owns rows S_m = [1024*m, 1024*(m+1)).

Phase A (Gram + adjacency + degrees): the scalar (activation) engine
preloads each PSUM bank with the fp16 column threshold
-(sq_j/2 - eps2/4); the tensor engine accumulates the bf16 Gram block
on top (start=False), issuing matmuls bank-interleaved across all 8
PSUM banks so accumulation chains of neighbouring output tiles overlap
(and each 128-wide weight load serves 8 matmuls); a custom fused DVE
op evicts `adj = (g >= rj_i)` as a bf16 0/1 tile while accumulating
the row-degree partial in the same pass (16 independent partials per
row chunk, combined by one tiny reduce), so degrees are ready the
moment the last eviction lands.  Three engines pipeline around the
PSUM banks: act fills, PE accumulates, vector evicts.

Phase C (label propagation): comp values are encoded as ORDINALS in
bf16 — index i maps to the i-th largest positive bf16 value (exactly
representable; products with {0,1} and max comparisons are exact) —
so min-index propagation becomes max propagation over bf16 data.
Per iteration: a 2KB AllGather shares each core's updated chunk, one
broadcast-DMA replicates the gathered [1, 8192] row to all 128
partitions, and a custom fused DVE op (mult + max-accumulate in one
pass) computes max_j adj[i,j] * n_j per 128-row chunk, halves chained
through the accumulator so compute overlaps the replication DMA.

Propagation runs a fixed 3 iterations; the host verifies the fixpoint
(iter2 == iter3 — the exact while-loop exit condition of the
reference) and falls back to a full numpy recomputation if it has not
converged (it has: this dataset converges after 2 iterations).  The
tiny O(N) label-numbering tail runs on the host.
"""

import numpy as np
import ml_dtypes

N = 8192
D = 256
NCORES = 8
NPC = N // NCORES          # 1024 rows per core
RCH = NPC // 128           # 8 row-chunks of 128 per core
HALF = N // 2              # 4096
EPS2 = 484.0               # 22.0**2
MIN_SAMPLES = 5
BIG = N
NITER = 3                  # fixpoint after 2 on this data; 3rd proves it

# Ordinal encoding: index i -> i-th largest positive bf16 (starting at 1.0).
# All values exact in bf16; decreasing in i; 0.0 = "no label" sentinel.
_ORD_BITS = (0x3F80 - np.arange(N, dtype=np.int64)).astype(np.uint16)
ORDS = _ORD_BITS.view(ml_dtypes.bfloat16).astype(np.float32)   # [N] f32, exact

_CACHE = {}


def _register_dve_op(name, spec):
    from concourse import dve_ops as dv
    from concourse.dve_spec import lower
    from concourse.dve_uop import DveOpSpec

    existing = [op for op in dv.OPS if op.name == name]
    if existing:
        return existing[0]
    op = dv.DveOp(name, spec, subdim=False, uops_sha={})
    dv.OPS.append(op)
    dv.CUSTOM_DVE_SPECS[name] = spec
    dv._SUB_OPCODE_FOR_NAME[name] = dv._CUSTOM_DVE_ROW_BASE + len(dv.OPS) - 1
    assert dv._SUB_OPCODE_FOR_NAME[name] < 0x20
    for ver in ("v3", "v4"):
        try:
            s = DveOpSpec(
                name=name,
                opcode=dv.get_dve_sub_opcode(name),
                uops=lower(spec, ver=ver),
                rd1_en=dv.has_src1(spec),
            )
            op.uops_sha[ver] = s.sha(ver)
        except Exception:
            pass
    return op


def _get_maxred_op():
    """Fused masked-max:  out = in0 * in1;  accum_out = max(s0, max_k out)
    (the production TENSOR_TENSOR_REDUCE crashes the exec unit on this
    runtime, the custom op works)."""
    if "maxred" in _CACHE:
        return _CACHE["maxred"]
    from concourse.dve_spec import Spec, Src0, Src1, C0, maxx

    def _ref(in0, in1, c0, c1, c2):
        b = (np.asarray(in0, np.float32) * np.asarray(in1, np.float32)).astype(
            np.float32)
        red = b.reshape(b.shape[0], -1).max(axis=-1, keepdims=True)
        return b, np.maximum(np.asarray(c0, np.float32), red)

    spec = Spec(body=Src0 * Src1, accum=maxx, accum_init=C0, reference=_ref)
    op = _register_dve_op("TT_MAXRED_DBSCAN_ANT", spec)
    _CACHE["maxred"] = op
    return op


def _get_evictdeg_op():
    """Fused adjacency eviction + degree accumulation:
        out       = (in0 >= s1)          (bf16 0/1 adjacency tile)
        accum_out = s0 + sum_k out[:, k] (running row degree, f32)"""
    if "evictdeg" in _CACHE:
        return _CACHE["evictdeg"]
    import operator
    from concourse.dve_spec import Spec, Src0, C0, C1

    def _ref(in0, in1, c0, c1, c2):
        b = (np.asarray(in0, np.float32) >= np.asarray(c1, np.float32))
        b = b.astype(np.float32)
        red = b.reshape(b.shape[0], -1).sum(axis=-1, keepdims=True)
        return b, np.asarray(c0, np.float32) + red

    spec = Spec(body=Src0 >= C1, accum=operator.add, accum_init=C0,
                reference=_ref)
    op = _register_dve_op("GE_DEGSUM_DBSCAN_ANT", spec)
    _CACHE["evictdeg"] = op
    return op


def _build_nc():
    import concourse.bass as bass
    import concourse.bacc as bacc
    import concourse.tile as tile
    import concourse.mybir as mybir

    f32 = mybir.dt.float32
    bf16 = mybir.dt.bfloat16
    Alu = mybir.AluOpType
    maxred = _get_maxred_op()
    evictdeg = _get_evictdeg_op()

    nc = bacc.Bacc("TRN2", target_bir_lowering=False, debug=False,
                   num_devices=NCORES)

    fp16 = mybir.dt.float16
    Act = mybir.ActivationFunctionType

    # ---- kernel I/O ----
    xt_d = nc.declare_dram_parameter("xt", [2, 128, N], bf16, isOutput=False)
    xo_d = nc.declare_dram_parameter("xtown", [2, 128, NPC], bf16, isOutput=False)
    cr_d = nc.declare_dram_parameter("crow", [1, N], fp16, isOutput=False)
    rj_d = nc.declare_dram_parameter("rj", [128, RCH], f32, isOutput=False)
    vv_d = nc.declare_dram_parameter("vv", [128, RCH], bf16, isOutput=False)

    deg_o = nc.declare_dram_parameter("deg", [128, RCH], f32, isOutput=True)
    bord_o = nc.declare_dram_parameter("bord", [128, RCH], f32, isOutput=True)
    c2_o = nc.declare_dram_parameter("comp2", [128, RCH], bf16, isOutput=True)
    c3_o = nc.declare_dram_parameter("comp3", [128, RCH], bf16, isOutput=True)

    with tile.TileContext(nc) as tc:
        with (
            tc.tile_pool(name="adj", bufs=1) as adjp,
            tc.tile_pool(name="misc", bufs=1) as misc,
            tc.tile_pool(name="dram", bufs=1, space="DRAM") as dram,
        ):
            adj = [adjp.tile([128, N], bf16, tag=f"adj{r}", name=f"adj{r}")
                   for r in range(RCH)]

            rj = misc.tile([128, RCH], f32, tag="rj")
            nc.sync.dma_start(rj[:], rj_d[:])
            vv = misc.tile([128, RCH], bf16, tag="vv")
            nc.sync.dma_start(vv[:], vv_d[:])

            deg_sb = misc.tile([128, RCH], f32, tag="deg")
            dp = misc.tile([128, 16], f32, tag="dp")      # per-tile degree partials
            core01 = misc.tile([128, RCH], bf16, tag="core01")
            comp0 = misc.tile([128, RCH], bf16, tag="comp0")
            zero = misc.tile([128, 1], f32, tag="zero")
            nc.vector.memset(zero[:], 0.0)

            # ---------------- phase A: G block + adjacency + degree ------
            with (
                tc.tile_pool(name="xt", bufs=1) as xtp,
                tc.tile_pool(name="psA", bufs=8, space=bass.MemorySpace.PSUM) as psA,
            ):
                xo0 = xtp.tile([128, NPC], bf16, tag="xo0")
                nc.gpsimd.dma_start(xo0[:], xo_d[0])
                xo1 = xtp.tile([128, NPC], bf16, tag="xo1")
                nc.gpsimd.dma_start(xo1[:], xo_d[1])
                # column threshold -(sq_j/2 - eps2/4), replicated to all
                # partitions (feeds the act-engine PSUM preload)
                cbt = xtp.tile([128, N], fp16, tag="cbt")
                nc.gpsimd.dma_start(cbt[:], cr_d[:].partition_broadcast(128))
                xt0 = xtp.tile([128, N], bf16, tag="xt0")
                xt1 = xtp.tile([128, N], bf16, tag="xt1")
                # DMA in column strips on two queues so r=0 matmuls start early
                for s in range(4):
                    sl = slice(s * 2048, (s + 1) * 2048)
                    nc.sync.dma_start(xt0[:, sl], xt_d[0][:, sl])
                    nc.scalar.dma_start(xt1[:, sl], xt_d[1][:, sl])

                for r in range(RCH):
                    # own-row lhsT: local column c = p*8 + r  ->  [:, r::8]
                    l0 = xo0[:, r::RCH]
                    l1 = xo1[:, r::RCH]
                    for grp in range(2):
                        gts = [psA.tile([128, 512], f32, tag="g", name="g")
                               for _ in range(8)]
                        sls = [slice(grp * 4096 + b * 512,
                                     grp * 4096 + (b + 1) * 512)
                               for b in range(8)]
                        # act engine preloads the column threshold into PSUM;
                        # matmuls accumulate the Gram block on top (no reset)
                        for b in range(8):
                            nc.scalar.activation(gts[b][:], cbt[:, sls[b]],
                                                 Act.Identity, bias=0.0,
                                                 scale=1.0)
                        # bank-interleaved: one weight load feeds 8 matmuls,
                        # and accumulation chains of distinct banks overlap
                        for b in range(8):
                            nc.tensor.matmul(gts[b][:], l0, xt0[:, sls[b]],
                                             start=False, stop=False,
                                             skip_group_check=True)
                        for b in range(8):
                            nc.tensor.matmul(gts[b][:], l1, xt1[:, sls[b]],
                                             start=False, stop=True,
                                             skip_group_check=True)
                        for b in range(8):
                            # adj = (g >= rj_i), per-tile degree partial
                            k = grp * 8 + b
                            nc.vector._custom_dve(
                                evictdeg,
                                out=adj[r][:, sls[b]],
                                in0=gts[b][:],
                                s0=zero[:, 0:1], s1=rj[:, r:r + 1],
                                accum_out=dp[:, k:k + 1],
                            )
                    # degree = sum of the 16 tile partials (tiny), then this
                    # chunk's core mask + ordinal label immediately
                    nc.vector.tensor_reduce(
                        out=deg_sb[:, r:r + 1], in_=dp[:],
                        axis=mybir.AxisListType.X, op=Alu.add)
                    nc.vector.tensor_scalar(
                        out=core01[:, r:r + 1], in0=deg_sb[:, r:r + 1],
                        scalar1=float(MIN_SAMPLES), scalar2=None, op0=Alu.is_ge)
                    nc.vector.tensor_mul(comp0[:, r:r + 1], core01[:, r:r + 1],
                                         vv[:, r:r + 1])
                    if r == RCH - 2:
                        # chunks 0..6 done: all-gather them now, hidden under
                        # the last row-chunk's compute
                        agi_a = dram.tile([128, RCH - 1], bf16, tag="agi_a")
                        nc.scalar.dma_start(agi_a[:], comp0[:, 0:RCH - 1])
                        ago_a = dram.tile([1, NPC * (RCH - 1)], bf16,
                                          tag="ago_a", addr_space="Shared")
                        nc.gpsimd.collective_compute(
                            "AllGather", Alu.bypass,
                            replica_groups=[list(range(NCORES))],
                            ins=[agi_a[:].opt()], outs=[ago_a[:].opt()],
                        )

            # ---------------- phase C: ordinal max propagation ------------
            with (
                tc.tile_pool(name="nbp", bufs=1) as nbp,
                tc.tile_pool(name="scrp", bufs=1) as scrp,
            ):
                comp_cur = comp0
                mh1 = None
                comp_iters = []
                for t in range(1, NITER + 1):
                    mh0 = misc.tile([128, RCH], f32, tag=f"mh0_{t}",
                                    name=f"mh0_{t}")
                    mh = misc.tile([128, RCH], f32, tag=f"mh_{t}",
                                   name=f"mh_{t}")
                    nbs = []
                    if t == 1:
                        # bulk chunks 0..6 were gathered under phase A; only
                        # the last chunk's 256B gather is serial here
                        agi_b = dram.tile([128, 1], bf16, tag="agi_b")
                        nc.scalar.dma_start(agi_b[:], comp0[:, RCH - 1:RCH])
                        ago_b = dram.tile([1, NPC], bf16, tag="ago_b",
                                          addr_space="Shared")
                        nc.gpsimd.collective_compute(
                            "AllGather", Alu.bypass,
                            replica_groups=[list(range(NCORES))],
                            ins=[agi_b[:].opt()], outs=[ago_b[:].opt()],
                        )
                        # strided assembly: nb[p, a*8 + q] = ago_a[a*7 + q]
                        # for q<7, ago_b[a] for q=7  (a = global row / 8)
                        NA = N // 16  # rows of the [*, 8] view per half
                        for h in range(2):
                            nb = nbp.tile([128, HALF], bf16, tag=f"nb{h}",
                                          name=f"nb1_{h}")
                            nbv = nb[:].rearrange("p (a b) -> p a b", b=RCH)
                            av = (ago_a[0:1, h * NA * 7:(h + 1) * NA * 7]
                                  .partition_broadcast(128)
                                  .rearrange("p o (a b) -> p (o a) b",
                                             b=RCH - 1))
                            nc.sync.dma_start(nbv[:, :, 0:RCH - 1], av)
                            bv = (ago_b[0:1, h * NA:(h + 1) * NA]
                                  .partition_broadcast(128)
                                  .rearrange("p o (a b) -> p (o a) b", b=1))
                            nc.sync.dma_start(nbv[:, :, RCH - 1:RCH], bv)
                            nbs.append(nb)
                    else:
                        # all-gather own chunk -> full ordinal vector
                        agi = dram.tile([128, RCH], bf16, tag=f"agi{t}",
                                        name=f"agi{t}")
                        nc.scalar.dma_start(agi[:], comp_cur[:])
                        ago = dram.tile([1, N], bf16, tag=f"ago{t}",
                                        name=f"ago{t}", addr_space="Shared")
                        nc.gpsimd.collective_compute(
                            "AllGather",
                            Alu.bypass,
                            replica_groups=[list(range(NCORES))],
                            ins=[agi[:].opt()],
                            outs=[ago[:].opt()],
                        )
                        # replicate gathered row to all partitions (per half)
                        for h in range(2):
                            nb = nbp.tile([128, HALF], bf16, tag=f"nb{h}",
                                          name=f"nb{t}_{h}")
                            nc.sync.dma_start(
                                nb[:],
                                ago[0:1, h * HALF:(h + 1) * HALF]
                                .partition_broadcast(128))
                            nbs.append(nb)
                    # fused mult+max per chunk, halves chained
                    for h in range(2):
                        for r in range(RCH):
                            scr = scrp.tile([128, HALF], bf16, tag="scr",
                                            name=f"scr{t}_{h}_{r}")
                            nc.vector._custom_dve(
                                maxred,
                                out=scr[:],
                                in0=adj[r][:, h * HALF:(h + 1) * HALF],
                                in1=nbs[h][:],
                                s0=(zero[:, 0:1] if h == 0
                                    else mh0[:, r:r + 1]),
                                accum_out=(mh0[:, r:r + 1] if h == 0
                                           else mh[:, r:r + 1]),
                            )
                    if t == 1:
                        mh1 = mh
                    compn = misc.tile([128, RCH], bf16, tag=f"comp{t}",
                                      name=f"comp{t}")
                    nc.vector.tensor_mul(compn[:], core01[:], mh[:])
                    comp_iters.append(compn)
                    comp_cur = compn

            # ---------------- outputs ------------------------------------
            nc.sync.dma_start(deg_o[:], deg_sb[:])
            nc.sync.dma_start(bord_o[:], mh1[:])
            nc.sync.dma_start(c2_o[:], comp_iters[1][:])
            nc.sync.dma_start(c3_o[:], comp_iters[2][:])

    nc.compile()
    return nc


def _prepare_inputs(X):
    X = np.ascontiguousarray(X, dtype=np.float32)
    sq = np.sum(X * X, axis=1, dtype=np.float32)          # [N]
    # adj  <=>  G >= (sq_i/2 - eps2/4) + (sq_j/2 - eps2/4)
    thr = sq * np.float32(0.5) - np.float32(EPS2 / 4.0)   # [N]

    xt_bf = X.T.astype(ml_dtypes.bfloat16)                # [256, 8192]
    xt = np.ascontiguousarray(xt_bf.reshape(2, 128, N))

    crow = (-thr).astype(np.float16).reshape(1, N)        # [1, 8192]

    ords_bf = ORDS.astype(ml_dtypes.bfloat16)             # exact
    in_maps = []
    for m in range(NCORES):
        rows = np.arange(m * NPC, (m + 1) * NPC)
        # local i = p*RCH + r  ->  [128, RCH] layout
        rows_pr = rows.reshape(128, RCH)
        in_maps.append({
            "xt": xt,
            "crow": crow,
            "xtown": np.ascontiguousarray(xt_bf[:, rows].reshape(2, 128, NPC)),
            "rj": np.ascontiguousarray(thr[rows_pr]),
            "vv": np.ascontiguousarray(ords_bf[rows_pr]),
        })
    return in_maps


def _decode_ords(vals_f32):
    """Map ordinal-encoded f32 values back to indices; 0.0 -> BIG."""
    vals = np.asarray(vals_f32, np.float32)
    bits = vals.astype(ml_dtypes.bfloat16).view(np.uint16).astype(np.int64)
    idx = 0x3F80 - bits
    out = np.where(vals == 0.0, BIG, idx)
    ok = (vals == 0.0) | ((idx >= 0) & (idx < N))
    return out, bool(ok.all())


def _host_finish(deg, bord, comp):
    """Exact numpy port of the reference's label-numbering tail."""
    idx = np.arange(N, dtype=np.int64)
    core = deg >= MIN_SAMPLES
    is_rep = core & (comp == idx)
    cid = np.cumsum(is_rep.astype(np.int64)) - 1
    comp_safe = np.minimum(comp, N - 1)
    core_label = np.where(core, cid[comp_safe], -1)
    first_core_nb = bord
    has_nb = first_core_nb < N
    nb_safe = np.minimum(first_core_nb, N - 1)
    border_label = np.where(has_nb, core_label[nb_safe], -1)
    return np.where(core, core_label, border_label).astype(np.int64)


def _host_fallback(X):
    """Full-precision numpy recomputation (only used if the device
    propagation has not reached the fixpoint, which does not happen)."""
    X = np.asarray(X, dtype=np.float32)
    sq = np.sum(X * X, axis=1, dtype=np.float32)
    G = X @ X.T
    d2 = sq[:, None] + sq[None, :] - 2.0 * G
    adj = d2 <= np.float32(EPS2)
    deg = adj.sum(1)
    core = deg >= MIN_SAMPLES
    idx = np.arange(N, dtype=np.int64)
    comp = np.where(core, idx, BIG)
    adjc = adj & core[None, :]
    while True:
        new = comp.copy()
        for s in range(0, N, 1024):
            cand = np.where(adjc[s:s + 1024], comp[None, :], BIG).min(1)
            new[s:s + 1024] = np.minimum(comp[s:s + 1024], cand)
        new = np.where(core, new, BIG)
        if (new == comp).all():
            break
        comp = new
    bord = np.where(adjc, idx[None, :], BIG).min(1)
    return _host_finish(deg.astype(np.int64), bord, comp)


def _flatten_out(arrs):
    """[8 cores][128, RCH] -> [8192] in global row order."""
    return np.concatenate([np.asarray(a, np.float32).reshape(-1) for a in arrs])


def _run_device(in_maps):
    from concourse import bass_utils
    if "nc" not in _CACHE:
        _CACHE["nc"] = _build_nc()
    res = bass_utils.run_bass_kernel_spmd(
        _CACHE["nc"], in_maps, list(range(NCORES)))
    return res.results


def kernel(X):
    in_maps = _prepare_inputs(X)
    results = _run_device(in_maps)

    deg = _flatten_out([results[m]["deg"] for m in range(NCORES)])
    vbord = _flatten_out([results[m]["bord"] for m in range(NCORES)])
    v2 = _flatten_out([results[m]["comp2"] for m in range(NCORES)])
    v3 = _flatten_out([results[m]["comp3"] for m in range(NCORES)])

    if not np.array_equal(v2, v3):
        return _host_fallback(X)

    comp, ok1 = _decode_ords(v3)
    bord, ok2 = _decode_ords(vbord)
    if not (ok1 and ok2):
        return _host_fallback(X)
    degi = np.rint(deg).astype(np.int64)
    return _host_finish(degi, bord, comp)


# revision 23
# speedup vs baseline: 3.0429x; 3.0429x over previous
"""DBSCAN (eps=22, min_samples=5) on X[8192, 256] float32, distributed
across 8 TRN2 NeuronCores via Bass/Tile.

Math (mirrors the jax reference):
  d2[i,j] = ||x_i||^2 + ||x_j||^2 - 2 (X X^T)[i,j]
  adj     = d2 <= eps^2
  core_i  = rowsum(adj) >= min_samples
  comp    = min-index label propagation over the core-core eps-graph
  labels  = component ids in scan order; border points attach to the
            min-index core neighbor; rest are noise (-1).

Sharding: core m owns rows S_m = [1024*m, 1024*(m+1)).

Phase A (Gram + adjacency + degrees): the scalar (activation) engine
preloads each PSUM bank with the fp16 column threshold
-(sq_j/2 - eps2/4); the tensor engine accumulates the bf16 Gram block
on top (start=False), issuing matmuls bank-interleaved across all 8
PSUM banks so accumulation chains of neighbouring output tiles overlap
(and each 128-wide weight load serves 8 matmuls); a custom fused DVE
op evicts `adj = (g >= rj_i)` as a bf16 0/1 tile while accumulating
the row-degree partial in the same pass (16 independent partials per
row chunk, combined by one tiny reduce), so degrees are ready the
moment the last eviction lands.  Three engines pipeline around the
PSUM banks: act fills, PE accumulates, vector evicts.

Phase C (label propagation): comp values are encoded as ORDINALS in
bf16 — index i maps to the i-th largest positive bf16 value (exactly
representable; products with {0,1} and max comparisons are exact) —
so min-index propagation becomes max propagation over bf16 data.
Per iteration: a 2KB AllGather shares each core's updated chunk, one
broadcast-DMA replicates the gathered [1, 8192] row to all 128
partitions, and a custom fused DVE op (mult + max-accumulate in one
pass) computes max_j adj[i,j] * n_j per 128-row chunk, halves chained
through the accumulator so compute overlaps the replication DMA.

Propagation runs a fixed 3 iterations; the host verifies the fixpoint
(iter2 == iter3 — the exact while-loop exit condition of the
reference) and falls back to a full numpy recomputation if it has not
converged (it has: this dataset converges after 2 iterations).  The
tiny O(N) label-numbering tail runs on the host.
"""

import numpy as np
import ml_dtypes

N = 8192
D = 256
NCORES = 8
NPC = N // NCORES          # 1024 rows per core
RCH = NPC // 128           # 8 row-chunks of 128 per core
HALF = N // 2              # 4096
EPS2 = 484.0               # 22.0**2
MIN_SAMPLES = 5
BIG = N
NITER = 3                  # fixpoint after 2 on this data; 3rd proves it

# Ordinal encoding: index i -> i-th largest positive bf16 (starting at 1.0).
# All values exact in bf16; decreasing in i; 0.0 = "no label" sentinel.
_ORD_BITS = (0x3F80 - np.arange(N, dtype=np.int64)).astype(np.uint16)
ORDS = _ORD_BITS.view(ml_dtypes.bfloat16).astype(np.float32)   # [N] f32, exact

_CACHE = {}


def _register_dve_op(name, spec):
    from concourse import dve_ops as dv
    from concourse.dve_spec import lower
    from concourse.dve_uop import DveOpSpec

    existing = [op for op in dv.OPS if op.name == name]
    if existing:
        return existing[0]
    op = dv.DveOp(name, spec, subdim=False, uops_sha={})
    dv.OPS.append(op)
    dv.CUSTOM_DVE_SPECS[name] = spec
    dv._SUB_OPCODE_FOR_NAME[name] = dv._CUSTOM_DVE_ROW_BASE + len(dv.OPS) - 1
    assert dv._SUB_OPCODE_FOR_NAME[name] < 0x20
    for ver in ("v3", "v4"):
        try:
            s = DveOpSpec(
                name=name,
                opcode=dv.get_dve_sub_opcode(name),
                uops=lower(spec, ver=ver),
                rd1_en=dv.has_src1(spec),
            )
            op.uops_sha[ver] = s.sha(ver)
        except Exception:
            pass
    return op


def _get_maxred_op():
    """Fused masked-max:  out = in0 * in1;  accum_out = max(s0, max_k out)
    (the production TENSOR_TENSOR_REDUCE crashes the exec unit on this
    runtime, the custom op works)."""
    if "maxred" in _CACHE:
        return _CACHE["maxred"]
    from concourse.dve_spec import Spec, Src0, Src1, C0, maxx

    def _ref(in0, in1, c0, c1, c2):
        b = (np.asarray(in0, np.float32) * np.asarray(in1, np.float32)).astype(
            np.float32)
        red = b.reshape(b.shape[0], -1).max(axis=-1, keepdims=True)
        return b, np.maximum(np.asarray(c0, np.float32), red)

    spec = Spec(body=Src0 * Src1, accum=maxx, accum_init=C0, reference=_ref)
    op = _register_dve_op("TT_MAXRED_DBSCAN_ANT", spec)
    _CACHE["maxred"] = op
    return op


def _get_evictdeg_op():
    """Fused adjacency eviction + degree accumulation:
        out       = (in0 >= s1)          (bf16 0/1 adjacency tile)
        accum_out = s0 + sum_k out[:, k] (running row degree, f32)"""
    if "evictdeg" in _CACHE:
        return _CACHE["evictdeg"]
    import operator
    from concourse.dve_spec import Spec, Src0, C0, C1

    def _ref(in0, in1, c0, c1, c2):
        b = (np.asarray(in0, np.float32) >= np.asarray(c1, np.float32))
        b = b.astype(np.float32)
        red = b.reshape(b.shape[0], -1).sum(axis=-1, keepdims=True)
        return b, np.asarray(c0, np.float32) + red

    spec = Spec(body=Src0 >= C1, accum=operator.add, accum_init=C0,
                reference=_ref)
    op = _register_dve_op("GE_DEGSUM_DBSCAN_ANT", spec)
    _CACHE["evictdeg"] = op
    return op


def _build_nc():
    import concourse.bass as bass
    import concourse.bacc as bacc
    import concourse.tile as tile
    import concourse.mybir as mybir

    f32 = mybir.dt.float32
    bf16 = mybir.dt.bfloat16
    Alu = mybir.AluOpType
    maxred = _get_maxred_op()
    evictdeg = _get_evictdeg_op()

    nc = bacc.Bacc("TRN2", target_bir_lowering=False, debug=False,
                   num_devices=NCORES)

    fp16 = mybir.dt.float16
    Act = mybir.ActivationFunctionType

    # ---- kernel I/O ----
    xt_d = nc.declare_dram_parameter("xt", [2, 128, N], bf16, isOutput=False)
    xo_d = nc.declare_dram_parameter("xtown", [2, 128, NPC], bf16, isOutput=False)
    cr_d = nc.declare_dram_parameter("crow", [1, N], fp16, isOutput=False)
    rj_d = nc.declare_dram_parameter("rj", [128, RCH], f32, isOutput=False)
    vv_d = nc.declare_dram_parameter("vv", [128, RCH], bf16, isOutput=False)

    deg_o = nc.declare_dram_parameter("deg", [128, RCH], f32, isOutput=True)
    bord_o = nc.declare_dram_parameter("bord", [128, RCH], f32, isOutput=True)
    c2_o = nc.declare_dram_parameter("comp2", [128, RCH], bf16, isOutput=True)
    c3_o = nc.declare_dram_parameter("comp3", [128, RCH], bf16, isOutput=True)

    with tile.TileContext(nc) as tc:
        with (
            tc.tile_pool(name="adj", bufs=1) as adjp,
            tc.tile_pool(name="misc", bufs=1) as misc,
            tc.tile_pool(name="dram", bufs=1, space="DRAM") as dram,
        ):
            adj = [adjp.tile([128, N], bf16, tag=f"adj{r}", name=f"adj{r}")
                   for r in range(RCH)]

            rj = misc.tile([128, RCH], f32, tag="rj")
            nc.sync.dma_start(rj[:], rj_d[:])
            vv = misc.tile([128, RCH], bf16, tag="vv")
            nc.sync.dma_start(vv[:], vv_d[:])

            deg_sb = misc.tile([128, RCH], f32, tag="deg")
            dp = misc.tile([128, 16], f32, tag="dp")      # per-tile degree partials
            core01 = misc.tile([128, RCH], bf16, tag="core01")
            comp0 = misc.tile([128, RCH], bf16, tag="comp0")
            zero = misc.tile([128, 1], f32, tag="zero")
            nc.vector.memset(zero[:], 0.0)

            # ---------------- phase A: G block + adjacency + degree ------
            with (
                tc.tile_pool(name="xt", bufs=1) as xtp,
                tc.tile_pool(name="psA", bufs=8, space=bass.MemorySpace.PSUM) as psA,
            ):
                xo0 = xtp.tile([128, NPC], bf16, tag="xo0")
                nc.gpsimd.dma_start(xo0[:], xo_d[0])
                xo1 = xtp.tile([128, NPC], bf16, tag="xo1")
                nc.gpsimd.dma_start(xo1[:], xo_d[1])
                # column threshold -(sq_j/2 - eps2/4), replicated to all
                # partitions (feeds the act-engine PSUM preload)
                cbt = xtp.tile([128, N], fp16, tag="cbt")
                nc.gpsimd.dma_start(cbt[:], cr_d[:].partition_broadcast(128))
                xt0 = xtp.tile([128, N], bf16, tag="xt0")
                xt1 = xtp.tile([128, N], bf16, tag="xt1")
                # DMA in column strips on two queues so r=0 matmuls start early
                for s in range(4):
                    sl = slice(s * 2048, (s + 1) * 2048)
                    nc.sync.dma_start(xt0[:, sl], xt_d[0][:, sl])
                    nc.scalar.dma_start(xt1[:, sl], xt_d[1][:, sl])

                for r in range(RCH):
                    # own-row lhsT: local column c = p*8 + r  ->  [:, r::8]
                    l0 = xo0[:, r::RCH]
                    l1 = xo1[:, r::RCH]
                    for grp in range(2):
                        gts = [psA.tile([128, 512], f32, tag="g", name="g")
                               for _ in range(8)]
                        sls = [slice(grp * 4096 + b * 512,
                                     grp * 4096 + (b + 1) * 512)
                               for b in range(8)]
                        # act engine preloads the column threshold into PSUM;
                        # matmuls accumulate the Gram block on top (no reset)
                        for b in range(8):
                            nc.scalar.activation(gts[b][:], cbt[:, sls[b]],
                                                 Act.Identity, bias=0.0,
                                                 scale=1.0)
                        # bank-interleaved: one weight load feeds 8 matmuls,
                        # and accumulation chains of distinct banks overlap
                        for b in range(8):
                            nc.tensor.matmul(gts[b][:], l0, xt0[:, sls[b]],
                                             start=False, stop=False,
                                             skip_group_check=True)
                        for b in range(8):
                            nc.tensor.matmul(gts[b][:], l1, xt1[:, sls[b]],
                                             start=False, stop=True,
                                             skip_group_check=True)
                        for b in range(8):
                            # adj = (g >= rj_i), per-tile degree partial
                            k = grp * 8 + b
                            nc.vector._custom_dve(
                                evictdeg,
                                out=adj[r][:, sls[b]],
                                in0=gts[b][:],
                                s0=zero[:, 0:1], s1=rj[:, r:r + 1],
                                accum_out=dp[:, k:k + 1],
                            )
                    # degree = sum of the 16 tile partials (tiny)
                    nc.vector.tensor_reduce(
                        out=deg_sb[:, r:r + 1], in_=dp[:],
                        axis=mybir.AxisListType.X, op=Alu.add)

            # ---------------- core mask + comp0 = core * ord_i ------------
            nc.vector.tensor_scalar(
                out=core01[:], in0=deg_sb[:], scalar1=float(MIN_SAMPLES),
                scalar2=None, op0=Alu.is_ge,
            )
            nc.vector.tensor_mul(comp0[:], core01[:], vv[:])

            # ---------------- phase C: ordinal max propagation ------------
            with (
                tc.tile_pool(name="nbp", bufs=1) as nbp,
                tc.tile_pool(name="scrp", bufs=1) as scrp,
            ):
                comp_cur = comp0
                mh1 = None
                comp_iters = []
                for t in range(1, NITER + 1):
                    # all-gather own chunk -> full ordinal vector
                    agi = dram.tile([128, RCH], bf16, tag=f"agi{t}",
                                    name=f"agi{t}")
                    nc.scalar.dma_start(agi[:], comp_cur[:])
                    ago = dram.tile([1, N], bf16, tag=f"ago{t}", name=f"ago{t}",
                                    addr_space="Shared")
                    nc.gpsimd.collective_compute(
                        "AllGather",
                        Alu.bypass,
                        replica_groups=[list(range(NCORES))],
                        ins=[agi[:].opt()],
                        outs=[ago[:].opt()],
                    )

                    mh0 = misc.tile([128, RCH], f32, tag=f"mh0_{t}",
                                    name=f"mh0_{t}")
                    mh = misc.tile([128, RCH], f32, tag=f"mh_{t}",
                                   name=f"mh_{t}")
                    # replicate gathered row to all partitions (one DMA per half)
                    nbs = []
                    for h in range(2):
                        nb = nbp.tile([128, HALF], bf16, tag=f"nb{h}",
                                      name=f"nb{t}_{h}")
                        nc.sync.dma_start(
                            nb[:],
                            ago[0:1, h * HALF:(h + 1) * HALF]
                            .partition_broadcast(128))
                        nbs.append(nb)
                    # fused mult+max per chunk, halves chained
                    for h in range(2):
                        for r in range(RCH):
                            scr = scrp.tile([128, HALF], bf16, tag="scr",
                                            name=f"scr{t}_{h}_{r}")
                            nc.vector._custom_dve(
                                maxred,
                                out=scr[:],
                                in0=adj[r][:, h * HALF:(h + 1) * HALF],
                                in1=nbs[h][:],
                                s0=(zero[:, 0:1] if h == 0
                                    else mh0[:, r:r + 1]),
                                accum_out=(mh0[:, r:r + 1] if h == 0
                                           else mh[:, r:r + 1]),
                            )
                    if t == 1:
                        mh1 = mh
                    compn = misc.tile([128, RCH], bf16, tag=f"comp{t}",
                                      name=f"comp{t}")
                    nc.vector.tensor_mul(compn[:], core01[:], mh[:])
                    comp_iters.append(compn)
                    comp_cur = compn

            # ---------------- outputs ------------------------------------
            nc.sync.dma_start(deg_o[:], deg_sb[:])
            nc.sync.dma_start(bord_o[:], mh1[:])
            nc.sync.dma_start(c2_o[:], comp_iters[1][:])
            nc.sync.dma_start(c3_o[:], comp_iters[2][:])

    nc.compile()
    return nc


def _prepare_inputs(X):
    X = np.ascontiguousarray(X, dtype=np.float32)
    sq = np.sum(X * X, axis=1, dtype=np.float32)          # [N]
    # adj  <=>  G >= (sq_i/2 - eps2/4) + (sq_j/2 - eps2/4)
    thr = sq * np.float32(0.5) - np.float32(EPS2 / 4.0)   # [N]

    xt_bf = X.T.astype(ml_dtypes.bfloat16)                # [256, 8192]
    xt = np.ascontiguousarray(xt_bf.reshape(2, 128, N))

    crow = (-thr).astype(np.float16).reshape(1, N)        # [1, 8192]

    ords_bf = ORDS.astype(ml_dtypes.bfloat16)             # exact
    in_maps = []
    for m in range(NCORES):
        rows = np.arange(m * NPC, (m + 1) * NPC)
        # local i = p*RCH + r  ->  [128, RCH] layout
        rows_pr = rows.reshape(128, RCH)
        in_maps.append({
            "xt": xt,
            "crow": crow,
            "xtown": np.ascontiguousarray(xt_bf[:, rows].reshape(2, 128, NPC)),
            "rj": np.ascontiguousarray(thr[rows_pr]),
            "vv": np.ascontiguousarray(ords_bf[rows_pr]),
        })
    return in_maps


def _decode_ords(vals_f32):
    """Map ordinal-encoded f32 values back to indices; 0.0 -> BIG."""
    vals = np.asarray(vals_f32, np.float32)
    bits = vals.astype(ml_dtypes.bfloat16).view(np.uint16).astype(np.int64)
    idx = 0x3F80 - bits
    out = np.where(vals == 0.0, BIG, idx)
    ok = (vals == 0.0) | ((idx >= 0) & (idx < N))
    return out, bool(ok.all())


def _host_finish(deg, bord, comp):
    """Exact numpy port of the reference's label-numbering tail."""
    idx = np.arange(N, dtype=np.int64)
    core = deg >= MIN_SAMPLES
    is_rep = core & (comp == idx)
    cid = np.cumsum(is_rep.astype(np.int64)) - 1
    comp_safe = np.minimum(comp, N - 1)
    core_label = np.where(core, cid[comp_safe], -1)
    first_core_nb = bord
    has_nb = first_core_nb < N
    nb_safe = np.minimum(first_core_nb, N - 1)
    border_label = np.where(has_nb, core_label[nb_safe], -1)
    return np.where(core, core_label, border_label).astype(np.int64)


def _host_fallback(X):
    """Full-precision numpy recomputation (only used if the device
    propagation has not reached the fixpoint, which does not happen)."""
    X = np.asarray(X, dtype=np.float32)
    sq = np.sum(X * X, axis=1, dtype=np.float32)
    G = X @ X.T
    d2 = sq[:, None] + sq[None, :] - 2.0 * G
    adj = d2 <= np.float32(EPS2)
    deg = adj.sum(1)
    core = deg >= MIN_SAMPLES
    idx = np.arange(N, dtype=np.int64)
    comp = np.where(core, idx, BIG)
    adjc = adj & core[None, :]
    while True:
        new = comp.copy()
        for s in range(0, N, 1024):
            cand = np.where(adjc[s:s + 1024], comp[None, :], BIG).min(1)
            new[s:s + 1024] = np.minimum(comp[s:s + 1024], cand)
        new = np.where(core, new, BIG)
        if (new == comp).all():
            break
        comp = new
    bord = np.where(adjc, idx[None, :], BIG).min(1)
    return _host_finish(deg.astype(np.int64), bord, comp)


def _flatten_out(arrs):
    """[8 cores][128, RCH] -> [8192] in global row order."""
    return np.concatenate([np.asarray(a, np.float32).reshape(-1) for a in arrs])


def _run_device(in_maps):
    from concourse import bass_utils
    if "nc" not in _CACHE:
        _CACHE["nc"] = _build_nc()
    res = bass_utils.run_bass_kernel_spmd(
        _CACHE["nc"], in_maps, list(range(NCORES)))
    return res.results


def kernel(X):
    in_maps = _prepare_inputs(X)
    results = _run_device(in_maps)

    deg = _flatten_out([results[m]["deg"] for m in range(NCORES)])
    vbord = _flatten_out([results[m]["bord"] for m in range(NCORES)])
    v2 = _flatten_out([results[m]["comp2"] for m in range(NCORES)])
    v3 = _flatten_out([results[m]["comp3"] for m in range(NCORES)])

    if not np.array_equal(v2, v3):
        return _host_fallback(X)

    comp, ok1 = _decode_ords(v3)
    bord, ok2 = _decode_ords(vbord)
    if not (ok1 and ok2):
        return _host_fallback(X)
    degi = np.rint(deg).astype(np.int64)
    return _host_finish(degi, bord, comp)


# revision 24
# speedup vs baseline: 3.1288x; 1.0282x over previous
"""DBSCAN (eps=22, min_samples=5) on X[8192, 256] float32, distributed
across 8 TRN2 NeuronCores via Bass/Tile.

Math (mirrors the jax reference):
  d2[i,j] = ||x_i||^2 + ||x_j||^2 - 2 (X X^T)[i,j]
  adj     = d2 <= eps^2
  core_i  = rowsum(adj) >= min_samples
  comp    = min-index label propagation over the core-core eps-graph
  labels  = component ids in scan order; border points attach to the
            min-index core neighbor; rest are noise (-1).

Sharding: core m owns rows S_m = [1024*m, 1024*(m+1)).

Phase A (Gram + adjacency + degrees): the scalar (activation) engine
preloads each PSUM bank with the fp16 column threshold
-(sq_j/2 - eps2/4); the tensor engine accumulates the bf16 Gram block
on top (start=False), issuing matmuls bank-interleaved across all 8
PSUM banks so accumulation chains of neighbouring output tiles overlap
(and each 128-wide weight load serves 8 matmuls); a custom fused DVE
op evicts `adj = (g >= rj_i)` as a bf16 0/1 tile while accumulating
the row-degree partial in the same pass (16 independent partials per
row chunk, combined by one tiny reduce), so degrees are ready the
moment the last eviction lands.  Three engines pipeline around the
PSUM banks: act fills, PE accumulates, vector evicts.

Phase C (label propagation): comp values are encoded as ORDINALS in
bf16 — index i maps to the i-th largest positive bf16 value (exactly
representable; products with {0,1} and max comparisons are exact) —
so min-index propagation becomes max propagation over bf16 data.
Per iteration: a 2KB AllGather shares each core's updated chunk, one
broadcast-DMA replicates the gathered [1, 8192] row to all 128
partitions, and a custom fused DVE op (mult + max-accumulate in one
pass) computes max_j adj[i,j] * n_j per 128-row chunk, halves chained
through the accumulator so compute overlaps the replication DMA.

Propagation runs a fixed 3 iterations; the host verifies the fixpoint
(iter2 == iter3 — the exact while-loop exit condition of the
reference) and falls back to a full numpy recomputation if it has not
converged (it has: this dataset converges after 2 iterations).  The
tiny O(N) label-numbering tail runs on the host.
"""

import numpy as np
import ml_dtypes

N = 8192
D = 256
NCORES = 8
NPC = N // NCORES          # 1024 rows per core
RCH = NPC // 128           # 8 row-chunks of 128 per core
HALF = N // 2              # 4096
EPS2 = 484.0               # 22.0**2
MIN_SAMPLES = 5
BIG = N
NITER = 3                  # fixpoint after 2 on this data; 3rd proves it

# Ordinal encoding: index i -> i-th largest positive bf16 (starting at 1.0).
# All values exact in bf16; decreasing in i; 0.0 = "no label" sentinel.
_ORD_BITS = (0x3F80 - np.arange(N, dtype=np.int64)).astype(np.uint16)
ORDS = _ORD_BITS.view(ml_dtypes.bfloat16).astype(np.float32)   # [N] f32, exact

_CACHE = {}


def _register_dve_op(name, spec):
    from concourse import dve_ops as dv
    from concourse.dve_spec import lower
    from concourse.dve_uop import DveOpSpec

    existing = [op for op in dv.OPS if op.name == name]
    if existing:
        return existing[0]
    op = dv.DveOp(name, spec, subdim=False, uops_sha={})
    dv.OPS.append(op)
    dv.CUSTOM_DVE_SPECS[name] = spec
    dv._SUB_OPCODE_FOR_NAME[name] = dv._CUSTOM_DVE_ROW_BASE + len(dv.OPS) - 1
    assert dv._SUB_OPCODE_FOR_NAME[name] < 0x20
    for ver in ("v3", "v4"):
        try:
            s = DveOpSpec(
                name=name,
                opcode=dv.get_dve_sub_opcode(name),
                uops=lower(spec, ver=ver),
                rd1_en=dv.has_src1(spec),
            )
            op.uops_sha[ver] = s.sha(ver)
        except Exception:
            pass
    return op


def _get_maxred_op():
    """Fused masked-max:  out = in0 * in1;  accum_out = max(s0, max_k out)
    (the production TENSOR_TENSOR_REDUCE crashes the exec unit on this
    runtime, the custom op works)."""
    if "maxred" in _CACHE:
        return _CACHE["maxred"]
    from concourse.dve_spec import Spec, Src0, Src1, C0, maxx

    def _ref(in0, in1, c0, c1, c2):
        b = (np.asarray(in0, np.float32) * np.asarray(in1, np.float32)).astype(
            np.float32)
        red = b.reshape(b.shape[0], -1).max(axis=-1, keepdims=True)
        return b, np.maximum(np.asarray(c0, np.float32), red)

    spec = Spec(body=Src0 * Src1, accum=maxx, accum_init=C0, reference=_ref)
    op = _register_dve_op("TT_MAXRED_DBSCAN_ANT", spec)
    _CACHE["maxred"] = op
    return op


def _get_evictdeg_op():
    """Fused adjacency eviction + degree accumulation:
        out       = (in0 >= s1)          (bf16 0/1 adjacency tile)
        accum_out = s0 + sum_k out[:, k] (running row degree, f32)"""
    if "evictdeg" in _CACHE:
        return _CACHE["evictdeg"]
    import operator
    from concourse.dve_spec import Spec, Src0, C0, C1

    def _ref(in0, in1, c0, c1, c2):
        b = (np.asarray(in0, np.float32) >= np.asarray(c1, np.float32))
        b = b.astype(np.float32)
        red = b.reshape(b.shape[0], -1).sum(axis=-1, keepdims=True)
        return b, np.asarray(c0, np.float32) + red

    spec = Spec(body=Src0 >= C1, accum=operator.add, accum_init=C0,
                reference=_ref)
    op = _register_dve_op("GE_DEGSUM_DBSCAN_ANT", spec)
    _CACHE["evictdeg"] = op
    return op


def _build_nc():
    import concourse.bass as bass
    import concourse.bacc as bacc
    import concourse.tile as tile
    import concourse.mybir as mybir

    f32 = mybir.dt.float32
    bf16 = mybir.dt.bfloat16
    Alu = mybir.AluOpType
    maxred = _get_maxred_op()
    evictdeg = _get_evictdeg_op()

    nc = bacc.Bacc("TRN2", target_bir_lowering=False, debug=False,
                   num_devices=NCORES)

    fp16 = mybir.dt.float16
    Act = mybir.ActivationFunctionType

    # ---- kernel I/O ----
    xt_d = nc.declare_dram_parameter("xt", [2, 128, N], bf16, isOutput=False)
    xo_d = nc.declare_dram_parameter("xtown", [2, 128, NPC], bf16, isOutput=False)
    cr_d = nc.declare_dram_parameter("crow", [1, N], fp16, isOutput=False)
    rj_d = nc.declare_dram_parameter("rj", [128, RCH], f32, isOutput=False)
    vv_d = nc.declare_dram_parameter("vv", [128, RCH], bf16, isOutput=False)

    deg_o = nc.declare_dram_parameter("deg", [128, RCH], f32, isOutput=True)
    bord_o = nc.declare_dram_parameter("bord", [128, RCH], f32, isOutput=True)
    c2_o = nc.declare_dram_parameter("comp2", [128, RCH], bf16, isOutput=True)
    c3_o = nc.declare_dram_parameter("comp3", [128, RCH], bf16, isOutput=True)

    with tile.TileContext(nc) as tc:
        with (
            tc.tile_pool(name="adj", bufs=1) as adjp,
            tc.tile_pool(name="misc", bufs=1) as misc,
            tc.tile_pool(name="dram", bufs=1, space="DRAM") as dram,
        ):
            adj = [adjp.tile([128, N], bf16, tag=f"adj{r}", name=f"adj{r}")
                   for r in range(RCH)]

            rj = misc.tile([128, RCH], f32, tag="rj")
            nc.sync.dma_start(rj[:], rj_d[:])
            vv = misc.tile([128, RCH], bf16, tag="vv")
            nc.sync.dma_start(vv[:], vv_d[:])

            deg_sb = misc.tile([128, RCH], f32, tag="deg")
            dp = misc.tile([128, 16], f32, tag="dp")      # per-tile degree partials
            core01 = misc.tile([128, RCH], bf16, tag="core01")
            comp0 = misc.tile([128, RCH], bf16, tag="comp0")
            zero = misc.tile([128, 1], f32, tag="zero")
            nc.vector.memset(zero[:], 0.0)

            # ---------------- phase A: G block + adjacency + degree ------
            with (
                tc.tile_pool(name="xt", bufs=1) as xtp,
                tc.tile_pool(name="psA", bufs=8, space=bass.MemorySpace.PSUM) as psA,
            ):
                # column threshold -(sq_j/2 - eps2/4), replicated to all
                # partitions (feeds the act-engine PSUM preload); first strip
                # ahead of xo so the first preload/matmul chain starts early
                cbt = xtp.tile([128, N], fp16, tag="cbt")
                nc.gpsimd.dma_start(cbt[:, 0:2048],
                                    cr_d[0:1, 0:2048].partition_broadcast(128))
                xo0 = xtp.tile([128, NPC], bf16, tag="xo0")
                nc.gpsimd.dma_start(xo0[:], xo_d[0])
                xo1 = xtp.tile([128, NPC], bf16, tag="xo1")
                nc.gpsimd.dma_start(xo1[:], xo_d[1])
                for s in range(1, 4):
                    sl = slice(s * 2048, (s + 1) * 2048)
                    nc.gpsimd.dma_start(cbt[:, sl],
                                        cr_d[0:1, sl].partition_broadcast(128))
                xt0 = xtp.tile([128, N], bf16, tag="xt0")
                xt1 = xtp.tile([128, N], bf16, tag="xt1")
                # DMA in column strips on two queues so r=0 matmuls start early
                for s in range(4):
                    sl = slice(s * 2048, (s + 1) * 2048)
                    nc.sync.dma_start(xt0[:, sl], xt_d[0][:, sl])
                    nc.scalar.dma_start(xt1[:, sl], xt_d[1][:, sl])

                for r in range(RCH):
                    # own-row lhsT: local column c = p*8 + r  ->  [:, r::8]
                    l0 = xo0[:, r::RCH]
                    l1 = xo1[:, r::RCH]
                    for grp in range(2):
                        gts = [psA.tile([128, 512], f32, tag="g", name="g")
                               for _ in range(8)]
                        sls = [slice(grp * 4096 + b * 512,
                                     grp * 4096 + (b + 1) * 512)
                               for b in range(8)]
                        # act engine preloads the column threshold into PSUM;
                        # matmuls accumulate the Gram block on top (no reset)
                        for b in range(8):
                            nc.scalar.activation(gts[b][:], cbt[:, sls[b]],
                                                 Act.Identity, bias=0.0,
                                                 scale=1.0)
                        # bank-interleaved: one weight load feeds 8 matmuls,
                        # and accumulation chains of distinct banks overlap
                        for b in range(8):
                            nc.tensor.matmul(gts[b][:], l0, xt0[:, sls[b]],
                                             start=False, stop=False,
                                             skip_group_check=True)
                        for b in range(8):
                            nc.tensor.matmul(gts[b][:], l1, xt1[:, sls[b]],
                                             start=False, stop=True,
                                             skip_group_check=True)
                        for b in range(8):
                            # adj = (g >= rj_i), per-tile degree partial
                            k = grp * 8 + b
                            nc.vector._custom_dve(
                                evictdeg,
                                out=adj[r][:, sls[b]],
                                in0=gts[b][:],
                                s0=zero[:, 0:1], s1=rj[:, r:r + 1],
                                accum_out=dp[:, k:k + 1],
                            )
                    # degree = sum of the 16 tile partials (tiny)
                    nc.vector.tensor_reduce(
                        out=deg_sb[:, r:r + 1], in_=dp[:],
                        axis=mybir.AxisListType.X, op=Alu.add)

            # ---------------- core mask + comp0 = core * ord_i ------------
            nc.vector.tensor_scalar(
                out=core01[:], in0=deg_sb[:], scalar1=float(MIN_SAMPLES),
                scalar2=None, op0=Alu.is_ge,
            )
            nc.vector.tensor_mul(comp0[:], core01[:], vv[:])

            # ---------------- phase C: ordinal max propagation ------------
            with (
                tc.tile_pool(name="nbp", bufs=1) as nbp,
                tc.tile_pool(name="scrp", bufs=1) as scrp,
            ):
                comp_cur = comp0
                mh1 = None
                comp_iters = []
                for t in range(1, NITER + 1):
                    # all-gather own chunk -> full ordinal vector
                    agi = dram.tile([128, RCH], bf16, tag=f"agi{t}",
                                    name=f"agi{t}")
                    nc.scalar.dma_start(agi[:], comp_cur[:])
                    ago = dram.tile([1, N], bf16, tag=f"ago{t}", name=f"ago{t}",
                                    addr_space="Shared")
                    nc.gpsimd.collective_compute(
                        "AllGather",
                        Alu.bypass,
                        replica_groups=[list(range(NCORES))],
                        ins=[agi[:].opt()],
                        outs=[ago[:].opt()],
                    )

                    mh0 = misc.tile([128, RCH], f32, tag=f"mh0_{t}",
                                    name=f"mh0_{t}")
                    mh = misc.tile([128, RCH], f32, tag=f"mh_{t}",
                                   name=f"mh_{t}")
                    # replicate gathered row to all partitions (one DMA per half)
                    nbs = []
                    for h in range(2):
                        nb = nbp.tile([128, HALF], bf16, tag=f"nb{h}",
                                      name=f"nb{t}_{h}")
                        nc.sync.dma_start(
                            nb[:],
                            ago[0:1, h * HALF:(h + 1) * HALF]
                            .partition_broadcast(128))
                        nbs.append(nb)
                    # fused mult+max per chunk, halves chained
                    for h in range(2):
                        for r in range(RCH):
                            scr = scrp.tile([128, HALF], bf16, tag="scr",
                                            name=f"scr{t}_{h}_{r}")
                            nc.vector._custom_dve(
                                maxred,
                                out=scr[:],
                                in0=adj[r][:, h * HALF:(h + 1) * HALF],
                                in1=nbs[h][:],
                                s0=(zero[:, 0:1] if h == 0
                                    else mh0[:, r:r + 1]),
                                accum_out=(mh0[:, r:r + 1] if h == 0
                                           else mh[:, r:r + 1]),
                            )
                    if t == 1:
                        mh1 = mh
                    compn = misc.tile([128, RCH], bf16, tag=f"comp{t}",
                                      name=f"comp{t}")
                    nc.vector.tensor_mul(compn[:], core01[:], mh[:])
                    comp_iters.append(compn)
                    comp_cur = compn

            # ---------------- outputs ------------------------------------
            nc.sync.dma_start(deg_o[:], deg_sb[:])
            nc.sync.dma_start(bord_o[:], mh1[:])
            nc.sync.dma_start(c2_o[:], comp_iters[1][:])
            nc.sync.dma_start(c3_o[:], comp_iters[2][:])

    nc.compile()
    return nc


def _prepare_inputs(X):
    X = np.ascontiguousarray(X, dtype=np.float32)
    sq = np.sum(X * X, axis=1, dtype=np.float32)          # [N]
    # adj  <=>  G >= (sq_i/2 - eps2/4) + (sq_j/2 - eps2/4)
    thr = sq * np.float32(0.5) - np.float32(EPS2 / 4.0)   # [N]

    xt_bf = X.T.astype(ml_dtypes.bfloat16)                # [256, 8192]
    xt = np.ascontiguousarray(xt_bf.reshape(2, 128, N))

    crow = (-thr).astype(np.float16).reshape(1, N)        # [1, 8192]

    ords_bf = ORDS.astype(ml_dtypes.bfloat16)             # exact
    in_maps = []
    for m in range(NCORES):
        rows = np.arange(m * NPC, (m + 1) * NPC)
        # local i = p*RCH + r  ->  [128, RCH] layout
        rows_pr = rows.reshape(128, RCH)
        in_maps.append({
            "xt": xt,
            "crow": crow,
            "xtown": np.ascontiguousarray(xt_bf[:, rows].reshape(2, 128, NPC)),
            "rj": np.ascontiguousarray(thr[rows_pr]),
            "vv": np.ascontiguousarray(ords_bf[rows_pr]),
        })
    return in_maps


def _decode_ords(vals_f32):
    """Map ordinal-encoded f32 values back to indices; 0.0 -> BIG."""
    vals = np.asarray(vals_f32, np.float32)
    bits = vals.astype(ml_dtypes.bfloat16).view(np.uint16).astype(np.int64)
    idx = 0x3F80 - bits
    out = np.where(vals == 0.0, BIG, idx)
    ok = (vals == 0.0) | ((idx >= 0) & (idx < N))
    return out, bool(ok.all())


def _host_finish(deg, bord, comp):
    """Exact numpy port of the reference's label-numbering tail."""
    idx = np.arange(N, dtype=np.int64)
    core = deg >= MIN_SAMPLES
    is_rep = core & (comp == idx)
    cid = np.cumsum(is_rep.astype(np.int64)) - 1
    comp_safe = np.minimum(comp, N - 1)
    core_label = np.where(core, cid[comp_safe], -1)
    first_core_nb = bord
    has_nb = first_core_nb < N
    nb_safe = np.minimum(first_core_nb, N - 1)
    border_label = np.where(has_nb, core_label[nb_safe], -1)
    return np.where(core, core_label, border_label).astype(np.int64)


def _host_fallback(X):
    """Full-precision numpy recomputation (only used if the device
    propagation has not reached the fixpoint, which does not happen)."""
    X = np.asarray(X, dtype=np.float32)
    sq = np.sum(X * X, axis=1, dtype=np.float32)
    G = X @ X.T
    d2 = sq[:, None] + sq[None, :] - 2.0 * G
    adj = d2 <= np.float32(EPS2)
    deg = adj.sum(1)
    core = deg >= MIN_SAMPLES
    idx = np.arange(N, dtype=np.int64)
    comp = np.where(core, idx, BIG)
    adjc = adj & core[None, :]
    while True:
        new = comp.copy()
        for s in range(0, N, 1024):
            cand = np.where(adjc[s:s + 1024], comp[None, :], BIG).min(1)
            new[s:s + 1024] = np.minimum(comp[s:s + 1024], cand)
        new = np.where(core, new, BIG)
        if (new == comp).all():
            break
        comp = new
    bord = np.where(adjc, idx[None, :], BIG).min(1)
    return _host_finish(deg.astype(np.int64), bord, comp)


def _flatten_out(arrs):
    """[8 cores][128, RCH] -> [8192] in global row order."""
    return np.concatenate([np.asarray(a, np.float32).reshape(-1) for a in arrs])


def _run_device(in_maps):
    from concourse import bass_utils
    if "nc" not in _CACHE:
        _CACHE["nc"] = _build_nc()
    res = bass_utils.run_bass_kernel_spmd(
        _CACHE["nc"], in_maps, list(range(NCORES)))
    return res.results


def kernel(X):
    in_maps = _prepare_inputs(X)
    results = _run_device(in_maps)

    deg = _flatten_out([results[m]["deg"] for m in range(NCORES)])
    vbord = _flatten_out([results[m]["bord"] for m in range(NCORES)])
    v2 = _flatten_out([results[m]["comp2"] for m in range(NCORES)])
    v3 = _flatten_out([results[m]["comp3"] for m in range(NCORES)])

    if not np.array_equal(v2, v3):
        return _host_fallback(X)

    comp, ok1 = _decode_ords(v3)
    bord, ok2 = _decode_ords(vbord)
    if not (ok1 and ok2):
        return _host_fallback(X)
    degi = np.rint(deg).astype(np.int64)
    return _host_finish(degi, bord, comp)
